# revision 1
# baseline (speedup 1.0000x reference)
"""Conv7x7(SAME) + LIF scan kernel for Trainium2, 8 NeuronCores.

Strategy:
- Shard H=512 spatially: core c owns output rows [64c, 64c+64). Host passes
  each core its 70-row input slab (64 + 3-row halo each side, zero padded),
  so no device-to-device communication is needed.
- Conv: 7x7 fp32 conv as 7 banded matmuls on the TensorEngine (band = the 7
  row-taps for one column-shift dx; column shifts realized as free-dim offsets
  into a width-padded SBUF tile). PSUM accumulates over dx. The two width
  halves run as col-tiled matmul pairs (tile_position) so M=64 doesn't waste
  the 128-wide PE array; output lands natively as [128, 256] = (half, row) x
  colchunk.
- LIF: bit-exact replication of the reference's per-op fp32 arithmetic on the
  VectorEngine: 6 ops per timestep on [128, 256] tiles.
    s = (i * 0.1) - i          (= -i_dec, exact negation)
    d = i - v
    v = (d * 0.1) + v          (= v_dec)
    z = (v - 1.0) > 0
    v = 0 where z              (copy_predicated reset)
    i = x_t - s                (= i_dec + x_t bitwise)
"""
import numpy as np
import concourse.bacc as bacc
import concourse.mybir as mybir
import concourse.tile as tile
from concourse.bass_utils import run_bass_kernel_spmd

T, H, WD, KK, PAD = 128, 512, 512, 7, 3
NCORES = 8
ROWS = H // NCORES            # 64 output rows per core
KP = ROWS + 2 * PAD           # 70 input rows per core
XB = 8                        # x tile buffers
ZB = 8                        # z staging buffers
NPS = 8                       # psum tiles in flight (PSUM = 8 banks)

_cached = None


def _build():
    global _cached
    if _cached is not None:
        return _cached

    f32 = mybir.dt.float32
    u32 = mybir.dt.uint32
    Alu = mybir.AluOpType

    nc = bacc.Bacc("TRN2", debug=False, num_devices=NCORES)
    xs_d = nc.dram_tensor("xs", (T, KP, WD), f32, kind="ExternalInput")
    bm_d = nc.dram_tensor("bm", (KP, KK * ROWS), f32, kind="ExternalInput")
    zs_d = nc.dram_tensor("zs", (T, ROWS, WD), f32, kind="ExternalOutput")

    with tile.TileContext(nc) as tc:
        with (
            tc.tile_pool(name="pool", bufs=1) as pool,
            tc.tile_pool(name="psum", bufs=1, space="PSUM") as psum,
        ):
            bm_t = pool.tile([KP, KK * ROWS], f32)
            nc.gpsimd.dma_start(bm_t[:], bm_d.ap())

            xts = [pool.tile([KP, WD + 2 * PAD], f32, name=f"xt{i}")
                   for i in range(XB)]
            for xt in xts:
                nc.gpsimd.memset(xt[:], 0.0)

            zts = [pool.tile([128, 256], f32, name=f"zt{i}") for i in range(ZB)]
            pss = [psum.tile([128, 256], f32, name=f"ps{i}") for i in range(NPS)]

            v_t = pool.tile([128, 256], f32)
            i_t = pool.tile([128, 256], f32)
            d_t = pool.tile([128, 256], f32)
            s_t = pool.tile([128, 256], f32)
            zero_t = pool.tile([128, 256], f32)
            nc.gpsimd.memset(v_t[:], 0.0)
            nc.gpsimd.memset(i_t[:], 0.0)
            nc.gpsimd.memset(zero_t[:], 0.0)

            for t in range(T):
                xt = xts[t % XB]
                nc.sync.dma_start(xt[:, PAD:PAD + WD], xs_d.ap()[t])
                ps = pss[t % NPS]
                for dx in range(KK):
                    for h in range(2):
                        nc.tensor.matmul(
                            ps[h * 64:(h + 1) * 64, :],
                            bm_t[:, dx * ROWS:(dx + 1) * ROWS],
                            xt[:, h * 256 + dx: h * 256 + dx + 256],
                            start=(dx == 0), stop=(dx == KK - 1),
                            tile_position=(0, h * 64),
                        )
                z_t = zts[t % ZB]
                # LIF step (all DVE, bit-exact vs reference order)
                nc.vector.scalar_tensor_tensor(
                    s_t[:], i_t[:], 0.1, i_t[:], Alu.mult, Alu.subtract)
                nc.vector.tensor_tensor(d_t[:], i_t[:], v_t[:], Alu.subtract)
                # psum read happens early so the bank frees for t+NPS
                nc.vector.tensor_tensor(i_t[:], ps[:], s_t[:], Alu.subtract)
                nc.vector.scalar_tensor_tensor(
                    v_t[:], d_t[:], 0.1, v_t[:], Alu.mult, Alu.add)
                nc.vector.tensor_scalar(
                    z_t[:], v_t[:], 1.0, 0.0, Alu.subtract, Alu.is_gt)
                nc.vector.copy_predicated(v_t[:], z_t[:].bitcast(u32), zero_t[:])

                nc.sync.dma_start(
                    zs_d.ap()[t].rearrange("r (h n) -> h r n", h=2), z_t[:])

    nc.compile()
    _cached = nc
    return nc


def _build_bmats(W):
    """bm[k, dx*64 + m] = W[dy=k-m, dx] for 0 <= k-m <= 6."""
    W = np.asarray(W, np.float32).reshape(KK, KK)
    bm = np.zeros((KP, KK * ROWS), np.float32)
    for dx in range(KK):
        for m in range(ROWS):
            for dy in range(KK):
                bm[m + dy, dx * ROWS + m] = W[dy, dx]
    return bm


def kernel(x, W):
    x = np.asarray(x, np.float32)
    nc = _build()
    bm = _build_bmats(W)
    xp = np.pad(x[:, 0], ((0, 0), (PAD, PAD), (0, 0)))  # [T, H+6, W]
    in_maps = []
    for c in range(NCORES):
        shard = np.ascontiguousarray(xp[:, c * ROWS: c * ROWS + KP, :])
        in_maps.append({"xs": shard, "bm": bm})
    res = run_bass_kernel_spmd(nc, in_maps, core_ids=list(range(NCORES)))
    z = np.concatenate([r["zs"] for r in res.results], axis=1)  # [T, H, W]
    return z.reshape(T, 1, H, WD).astype(np.float32)



# revision 3
# speedup vs baseline: 2.2103x; 2.2103x over previous
"""Conv7x7(SAME) + LIF scan kernel for Trainium2, 8 NeuronCores.

Strategy (v2):
- Shard W=512 across cores: core c owns output cols [64c, 64c+64), receives a
  70-col slab (3-col halo each side, zero-padded) over all 512 rows and all
  128 timesteps, H-padded to 518 rows, laid out [518, 128, 70] in DRAM.
- Conv: contract over H on the TensorEngine. Stationary = banded matrix
  B[k, m] = W'[k-m, dx] mapping 128 input rows -> 122 output rows; the 512
  output rows split into 5 blocks at stride 122 (last block 24 valid rows).
  All 5 blocks merge into ONE matmul per tap via a 3D moving AP
  [128, (block, 64)], psum [122, 5*64=320]. 7 dx taps accumulate in PSUM.
- Precision: fp32 matmul costs 4 cycles/row; fp16/bf16 cost 1. The LIF spike
  cascade needs ~fp32 conv precision (bf16/tf32/fp32r all flip too many
  spikes), so the conv runs as a 3-term fp16 hi/lo decomposition
  (wh@xh + wh@xl + wl@xh), measured at ~1.5e-7 max abs error vs fp32.
  21 matmuls/step of ap 320 ~= 2.8us/step on the PE.
- LIF (4 DVE ops/step, tracking j = 0.1*i so the 0.1 folds into W'):
    u = 0.9*v + j ; m = (u <= 1) [bf16, the DMA'd output] ; v = u*m ;
    j = 0.9*j + psum. Host computes z = 1 - m.
- Input preloads in 16-step chunks (ring of 6); output masks batch 8 steps
  per DMA as raw [122, 2560] mega-tiles the host unscrambles.
"""
import numpy as np
import concourse.bacc as bacc
import concourse.mybir as mybir
import concourse.tile as tile
from concourse.bass_utils import run_bass_kernel_spmd

T, H, WD, KK, PAD = 128, 512, 512, 7, 3
NCORES = 8
COLS = WD // NCORES           # 64 output cols per core
KP = COLS + 2 * PAD           # 70 input cols per core
HP = H + 2 * PAD              # 518 padded rows
NB = 5                        # row blocks per step
BS = 122                      # output rows per block (contract 128, 7-tap)
CK = 16                       # timesteps per input chunk
NCHUNK = T // CK
RING = 6                      # resident chunk ring
GB = 8                        # timesteps per output mega-DMA
NG = T // GB

_cached = None


def _build():
    global _cached
    if _cached is not None:
        return _cached

    f32 = mybir.dt.float32
    f16 = mybir.dt.float16
    bf16 = mybir.dt.bfloat16
    Alu = mybir.AluOpType

    nc = bacc.Bacc("TRN2", debug=False, num_devices=NCORES)
    xh_d = nc.dram_tensor("xh", (HP, T, KP), f16, kind="ExternalInput")
    xl_d = nc.dram_tensor("xl", (HP, T, KP), f16, kind="ExternalInput")
    bmh_d = nc.dram_tensor("bmh", (128, KK * BS), f16, kind="ExternalInput")
    bml_d = nc.dram_tensor("bml", (128, KK * BS), f16, kind="ExternalInput")
    ms_d = nc.dram_tensor("ms", (NG, BS, GB * NB * 64), bf16,
                          kind="ExternalOutput")

    CW = CK * KP              # 1120 cols per block in a chunk tile

    with tile.TileContext(nc) as tc:
        with (
            tc.tile_pool(name="pool", bufs=1) as pool,
            tc.tile_pool(name="psum", bufs=1, space="PSUM") as psum,
        ):
            bmh_t = pool.tile([128, KK * BS], f16, name="bmh")
            bml_t = pool.tile([128, KK * BS], f16, name="bml")
            nc.sync.dma_start(bmh_t[:], bmh_d.ap())
            nc.sync.dma_start(bml_t[:], bml_d.ap())

            xhc = [pool.tile([128, NB * CW], f16, name=f"xh{r}")
                   for r in range(RING)]
            xlc = [pool.tile([128, NB * CW], f16, name=f"xl{r}")
                   for r in range(RING)]
            # block 4 reads rows 488..518 only; zero its region once — DMAs
            # overwrite [0:30, block4] but never [30:, block4], which stays
            # zero across ring reuse.
            for xt in xhc + xlc:
                nc.gpsimd.memset(xt[:, 4 * CW:5 * CW], 0.0)

            u_t = pool.tile([128, NB * 64], f32, name="u")
            v_t = pool.tile([128, NB * 64], f32, name="v")
            j_t = pool.tile([128, NB * 64], f32, name="j")
            nc.vector.memset(v_t[:], 0.0)
            nc.vector.memset(j_t[:], 0.0)

            mg = [pool.tile([128, GB * NB * 64], bf16, name=f"mg{i}")
                  for i in range(2)]
            pss = [psum.tile([128, NB * 64], f32, name=f"ps{i}")
                   for i in range(8)]

            def load_chunk(ck):
                r = ck % RING
                for dst, src in ((xhc[r], xh_d), (xlc[r], xl_d)):
                    for b in range(NB):
                        nrows = min(128, HP - BS * b)
                        nc.sync.dma_start(
                            dst[0:nrows, b * CW:(b + 1) * CW]
                            .rearrange("p (t q) -> p t q", q=KP),
                            src.ap()[BS * b:BS * b + nrows,
                                     ck * CK:(ck + 1) * CK, :])

            for ck in range(4):
                load_chunk(ck)

            for t in range(T):
                ck, tl = divmod(t, CK)
                if tl == 0 and ck + 4 < NCHUNK:
                    load_chunk(ck + 4)
                r = ck % RING
                mvh = xhc[r][:, :].rearrange(
                    "p (b t q) -> p b t q", b=NB, t=CK)
                mvl = xlc[r][:, :].rearrange(
                    "p (b t q) -> p b t q", b=NB, t=CK)
                ps = pss[t % 8]
                n = 0
                for dx in range(KK):
                    for bm_t, mv in ((bmh_t, mvh), (bmh_t, mvl),
                                     (bml_t, mvh)):
                        nc.tensor.matmul(
                            ps[0:BS, :],
                            bm_t[:, dx * BS:(dx + 1) * BS],
                            mv[:, :, tl:tl + 1, dx:dx + 64],
                            start=(n == 0), stop=(n == 3 * KK - 1),
                        )
                        n += 1

                msl = mg[(t // GB) % 2][0:BS, (t % GB) * 320:(t % GB + 1) * 320]
                nc.vector.scalar_tensor_tensor(
                    u_t[0:BS, :], v_t[0:BS, :], 0.9, j_t[0:BS, :],
                    Alu.mult, Alu.add)
                nc.vector.tensor_scalar(
                    msl, u_t[0:BS, :], 1.0, None, Alu.is_le)
                nc.vector.tensor_tensor(
                    v_t[0:BS, :], u_t[0:BS, :], msl, Alu.mult)
                nc.vector.scalar_tensor_tensor(
                    j_t[0:BS, :], j_t[0:BS, :], 0.9, ps[0:BS, :],
                    Alu.mult, Alu.add)

                if t % GB == GB - 1:
                    nc.sync.dma_start(ms_d.ap()[t // GB],
                                      mg[(t // GB) % 2][0:BS, :])

    nc.compile()
    _cached = nc
    return nc


def _bands(Wq):
    bm = np.zeros((128, KK * BS), np.float32)
    m = np.arange(BS)
    for dx in range(KK):
        for dy in range(KK):
            bm[m + dy, dx * BS + m] = Wq[dy, dx]
    return bm.astype(np.float16)


def kernel(x, W):
    x = np.asarray(x, np.float32)
    nc = _build()

    W01 = (np.float32(0.1) * np.asarray(W, np.float32).reshape(KK, KK))
    wh = W01.astype(np.float16)
    wl = (W01 - wh.astype(np.float32)).astype(np.float16)
    bmh = _bands(wh.astype(np.float32))
    bml = _bands(wl.astype(np.float32))

    in_maps = []
    for c in range(NCORES):
        slab = np.zeros((T, HP, KP), np.float32)
        lo = COLS * c - PAD
        s0, s1 = max(0, lo), min(WD, lo + KP)
        slab[:, PAD:PAD + H, s0 - lo:s0 - lo + s1 - s0] = x[:, 0, :, s0:s1]
        xs = np.ascontiguousarray(slab.transpose(1, 0, 2))   # [518, 128, 70]
        xh = xs.astype(np.float16)
        xl = (xs - xh.astype(np.float32)).astype(np.float16)
        in_maps.append({"xh": xh, "xl": xl, "bmh": bmh, "bml": bml})

    res = run_bass_kernel_spmd(nc, in_maps, core_ids=list(range(NCORES)))

    z = np.empty((T, H, WD), np.float32)
    for c in range(NCORES):
        ms = np.asarray(res.results[c]["ms"]).astype(np.float32)
        # [NG, BS, GB*NB*64] -> [t, block, row, w]
        m4 = ms.reshape(NG, BS, GB, NB, 64).transpose(0, 2, 3, 1, 4)
        m4 = m4.reshape(T, NB, BS, 64)
        zc = z[:, :, COLS * c:COLS * (c + 1)]
        zc[:, 0:4 * BS, :] = m4[:, 0:4].reshape(T, 4 * BS, 64)
        zc[:, 4 * BS:H, :] = m4[:, 4, 0:H - 4 * BS, :]
    return (np.float32(1.0) - z).reshape(T, 1, H, WD)


# revision 7
# speedup vs baseline: 2.2137x; 1.0016x over previous
"""Conv7x7(SAME) + LIF scan kernel for Trainium2, 8 NeuronCores.

Strategy (v2):
- Shard W=512 across cores: core c owns output cols [64c, 64c+64), receives a
  70-col slab (3-col halo each side, zero-padded) over all 512 rows and all
  128 timesteps, H-padded to 518 rows, laid out [518, 128, 70] in DRAM.
- Conv: contract over H on the TensorEngine. Stationary = banded matrix
  B[k, m] = W'[k-m, dx] mapping 128 input rows -> 122 output rows; the 512
  output rows split into 5 blocks at stride 122 (last block 24 valid rows).
  All 5 blocks merge into ONE matmul per tap via a 3D moving AP
  [128, (block, 64)], psum [122, 5*64=320]. 7 dx taps accumulate in PSUM.
- Precision: fp32 matmul costs 4 cycles/row; fp16/bf16 cost 1. The LIF spike
  cascade needs ~fp32 conv precision (bf16/tf32/fp32r all flip too many
  spikes), so the conv runs as a 3-term fp16 hi/lo decomposition
  (wh@xh + wh@xl + wl@xh), measured at ~1.5e-7 max abs error vs fp32.
  21 matmuls/step of ap 320 ~= 2.8us/step on the PE.
- LIF (4 DVE ops/step, tracking j = 0.1*i so the 0.1 folds into W'):
    u = 0.9*v + j ; m = (u <= 1) [bf16, the DMA'd output] ; v = u*m ;
    j = 0.9*j + psum. Host computes z = 1 - m.
- Input preloads in 16-step chunks (ring of 6); output masks batch 8 steps
  per DMA as raw [122, 2560] mega-tiles the host unscrambles.
"""
import numpy as np
import concourse.bacc as bacc
import concourse.mybir as mybir
import concourse.tile as tile
from concourse.bass_utils import run_bass_kernel_spmd

T, H, WD, KK, PAD = 128, 512, 512, 7, 3
NCORES = 8
COLS = WD // NCORES           # 64 output cols per core
KP = COLS + 2 * PAD           # 70 input cols per core
HP = H + 2 * PAD              # 518 padded rows
NB = 5                        # row blocks per step
BS = 122                      # output rows per block (contract 128, 7-tap)
CK = 16                       # max timesteps per input chunk (tile capacity)
CHUNKS = [2, 2, 4, 8] + [16] * 7          # sizes; small head to start PE early
CSTART = [sum(CHUNKS[:i]) for i in range(len(CHUNKS))]
RING = 6                      # resident chunk ring
GB = 4                        # timesteps per output mega-DMA
NG = T // GB

_cached = None


def _build():
    global _cached
    if _cached is not None:
        return _cached

    f32 = mybir.dt.float32
    f16 = mybir.dt.float16
    bf16 = mybir.dt.bfloat16
    Alu = mybir.AluOpType

    nc = bacc.Bacc("TRN2", debug=False, num_devices=NCORES)
    xh_d = nc.dram_tensor("xh", (HP, T, KP), f16, kind="ExternalInput")
    xl_d = nc.dram_tensor("xl", (HP, T, KP), f16, kind="ExternalInput")
    bmh_d = nc.dram_tensor("bmh", (128, KK * BS), f16, kind="ExternalInput")
    bml_d = nc.dram_tensor("bml", (128, KK * BS), f16, kind="ExternalInput")
    ms_d = nc.dram_tensor("ms", (NG, BS, GB * NB * 64), bf16,
                          kind="ExternalOutput")

    CW = CK * KP              # 1120 cols per block in a chunk tile

    with tile.TileContext(nc) as tc:
        with (
            tc.tile_pool(name="pool", bufs=1) as pool,
            tc.tile_pool(name="psum", bufs=1, space="PSUM") as psum,
        ):
            bmh_t = pool.tile([128, KK * BS], f16, name="bmh")
            bml_t = pool.tile([128, KK * BS], f16, name="bml")
            nc.sync.dma_start(bmh_t[:], bmh_d.ap())
            nc.sync.dma_start(bml_t[:], bml_d.ap())

            xhc = [pool.tile([128, NB * CW], f16, name=f"xh{r}")
                   for r in range(RING)]
            xlc = [pool.tile([128, NB * CW], f16, name=f"xl{r}")
                   for r in range(RING)]
            # block 4 reads rows 488..518 only; zero its region once — DMAs
            # overwrite [0:30, block4] but never [30:, block4], which stays
            # zero across ring reuse.
            for xt in xhc + xlc:
                nc.gpsimd.memset(xt[:, 4 * CW:5 * CW], 0.0)

            u_t = pool.tile([128, NB * 64], f32, name="u")
            v_t = pool.tile([128, NB * 64], f32, name="v")
            j_t = pool.tile([128, NB * 64], f32, name="j")
            nc.vector.memset(v_t[:], 0.0)
            nc.vector.memset(j_t[:], 0.0)

            mg = [pool.tile([128, GB * NB * 64], bf16, name=f"mg{i}")
                  for i in range(2)]
            pss = [psum.tile([128, NB * 64], f32, name=f"ps{i}")
                   for i in range(8)]

            in_eng = [nc.sync, nc.scalar]
            n_in = [0]

            def load_chunk(ci):
                r = ci % RING
                sz = CHUNKS[ci]
                t0 = CSTART[ci]
                for dst, src in ((xhc[r], xh_d), (xlc[r], xl_d)):
                    for b in range(NB):
                        nrows = min(128, HP - BS * b)
                        eng = in_eng[n_in[0] % len(in_eng)]
                        n_in[0] += 1
                        eng.dma_start(
                            dst[0:nrows, b * CW:b * CW + sz * KP]
                            .rearrange("p (t q) -> p t q", q=KP),
                            src.ap()[BS * b:BS * b + nrows, t0:t0 + sz, :])

            for ci in range(3):
                load_chunk(ci)

            # PE pstate warmup: harmless self-matmuls while preload streams in
            for i in range(16):
                nc.tensor.matmul(pss[7][0:BS, :], bmh_t[:, 0:BS],
                                 bmh_t[:, 0:320], start=True, stop=True)

            step2chunk = []
            for ci, sz in enumerate(CHUNKS):
                step2chunk += [(ci, tl) for tl in range(sz)]

            for t in range(T):
                ck, tl = step2chunk[t]
                if tl == 0 and ck + 3 < len(CHUNKS):
                    load_chunk(ck + 3)
                r = ck % RING
                mvh = xhc[r][:, :].rearrange(
                    "p (b t q) -> p b t q", b=NB, t=CK)
                mvl = xlc[r][:, :].rearrange(
                    "p (b t q) -> p b t q", b=NB, t=CK)
                ps = pss[t % 8]
                n = 0
                for dx in range(KK):
                    for bm_t, mv in ((bmh_t, mvh), (bmh_t, mvl),
                                     (bml_t, mvh)):
                        nc.tensor.matmul(
                            ps[0:BS, :],
                            bm_t[:, dx * BS:(dx + 1) * BS],
                            mv[:, :, tl:tl + 1, dx:dx + 64],
                            start=(n == 0), stop=(n == 3 * KK - 1),
                        )
                        n += 1

                msl = mg[(t // GB) % 2][0:BS, (t % GB) * 320:(t % GB + 1) * 320]
                nc.vector.scalar_tensor_tensor(
                    u_t[0:BS, :], v_t[0:BS, :], 0.9, j_t[0:BS, :],
                    Alu.mult, Alu.add)
                nc.vector.tensor_scalar(
                    msl, u_t[0:BS, :], 1.0, None, Alu.is_le)
                nc.vector.tensor_tensor(
                    v_t[0:BS, :], u_t[0:BS, :], msl, Alu.mult)
                nc.vector.scalar_tensor_tensor(
                    j_t[0:BS, :], j_t[0:BS, :], 0.9, ps[0:BS, :],
                    Alu.mult, Alu.add)

                if t % GB == GB - 1:
                    g = t // GB
                    nc.gpsimd.dma_start(ms_d.ap()[g], mg[g % 2][0:BS, :])

    nc.compile()
    _cached = nc
    return nc


def _bands(Wq):
    bm = np.zeros((128, KK * BS), np.float32)
    m = np.arange(BS)
    for dx in range(KK):
        for dy in range(KK):
            bm[m + dy, dx * BS + m] = Wq[dy, dx]
    return bm.astype(np.float16)


def kernel(x, W):
    x = np.asarray(x, np.float32)
    nc = _build()

    W01 = (np.float32(0.1) * np.asarray(W, np.float32).reshape(KK, KK))
    wh = W01.astype(np.float16)
    wl = (W01 - wh.astype(np.float32)).astype(np.float16)
    bmh = _bands(wh.astype(np.float32))
    bml = _bands(wl.astype(np.float32))

    in_maps = []
    for c in range(NCORES):
        slab = np.zeros((T, HP, KP), np.float32)
        lo = COLS * c - PAD
        s0, s1 = max(0, lo), min(WD, lo + KP)
        slab[:, PAD:PAD + H, s0 - lo:s0 - lo + s1 - s0] = x[:, 0, :, s0:s1]
        xs = np.ascontiguousarray(slab.transpose(1, 0, 2))   # [518, 128, 70]
        xh = xs.astype(np.float16)
        xl = (xs - xh.astype(np.float32)).astype(np.float16)
        in_maps.append({"xh": xh, "xl": xl, "bmh": bmh, "bml": bml})

    res = run_bass_kernel_spmd(nc, in_maps, core_ids=list(range(NCORES)))

    z = np.empty((T, H, WD), np.float32)
    for c in range(NCORES):
        ms = np.asarray(res.results[c]["ms"]).astype(np.float32)
        # [NG, BS, GB*NB*64] -> [t, block, row, w]
        m4 = ms.reshape(NG, BS, GB, NB, 64).transpose(0, 2, 3, 1, 4)
        m4 = m4.reshape(T, NB, BS, 64)
        zc = z[:, :, COLS * c:COLS * (c + 1)]
        zc[:, 0:4 * BS, :] = m4[:, 0:4].reshape(T, 4 * BS, 64)
        zc[:, 4 * BS:H, :] = m4[:, 4, 0:H - 4 * BS, :]
    return (np.float32(1.0) - z).reshape(T, 1, H, WD)


# revision 8
# speedup vs baseline: 2.2529x; 1.0177x over previous
"""Conv7x7(SAME) + LIF scan kernel for Trainium2, 8 NeuronCores.

Strategy (v2):
- Shard W=512 across cores: core c owns output cols [64c, 64c+64), receives a
  70-col slab (3-col halo each side, zero-padded) over all 512 rows and all
  128 timesteps, H-padded to 518 rows, laid out [518, 128, 70] in DRAM.
- Conv: contract over H on the TensorEngine. Stationary = banded matrix
  B[k, m] = W'[k-m, dx] mapping 128 input rows -> 122 output rows; the 512
  output rows split into 5 blocks at stride 122 (last block 24 valid rows).
  All 5 blocks merge into ONE matmul per tap via a 3D moving AP
  [128, (block, 64)], psum [122, 5*64=320]. 7 dx taps accumulate in PSUM.
- Precision: fp32 matmul costs 4 cycles/row; fp16/bf16 cost 1. The LIF spike
  cascade needs ~fp32 conv precision (bf16/tf32/fp32r all flip too many
  spikes), so the conv runs as a 3-term fp16 hi/lo decomposition
  (wh@xh + wh@xl + wl@xh), measured at ~1.5e-7 max abs error vs fp32.
  21 matmuls/step of ap 320 ~= 2.8us/step on the PE.
- LIF (4 DVE ops/step, tracking j = 0.1*i so the 0.1 folds into W'):
    u = 0.9*v + j ; m = (u <= 1) [bf16, the DMA'd output] ; v = u*m ;
    j = 0.9*j + psum. Host computes z = 1 - m.
- Input preloads in 16-step chunks (ring of 6); output masks batch 8 steps
  per DMA as raw [122, 2560] mega-tiles the host unscrambles.
"""
import numpy as np
import concourse.bacc as bacc
import concourse.mybir as mybir
import concourse.tile as tile
from concourse.bass_utils import run_bass_kernel_spmd

T, H, WD, KK, PAD = 128, 512, 512, 7, 3
NCORES = 8
COLS = WD // NCORES           # 64 output cols per core
KP = COLS + 2 * PAD           # 70 input cols per core
HP = H + 2 * PAD              # 518 padded rows
NB = 5                        # row blocks per step
BS = 122                      # output rows per block (contract 128, 7-tap)
CK = 16                       # max timesteps per input chunk (tile capacity)
CHUNKS = [4, 4, 8] + [16] * 7             # sizes; small head to start PE early
CSTART = [sum(CHUNKS[:i]) for i in range(len(CHUNKS))]
RING = 6                      # resident chunk ring
GB = 4                        # timesteps per output mega-DMA
NG = T // GB

_cached = None


def _build():
    global _cached
    if _cached is not None:
        return _cached

    f32 = mybir.dt.float32
    f16 = mybir.dt.float16
    bf16 = mybir.dt.bfloat16
    Alu = mybir.AluOpType

    nc = bacc.Bacc("TRN2", debug=False, num_devices=NCORES)
    xh_d = nc.dram_tensor("xh", (HP, T, KP), f16, kind="ExternalInput")
    xl_d = nc.dram_tensor("xl", (HP, T, KP), f16, kind="ExternalInput")
    bmh_d = nc.dram_tensor("bmh", (128, KK * BS), f16, kind="ExternalInput")
    bml_d = nc.dram_tensor("bml", (128, KK * BS), f16, kind="ExternalInput")
    ms_d = nc.dram_tensor("ms", (NG, BS, GB * NB * 64), bf16,
                          kind="ExternalOutput")

    CW = CK * KP              # 1120 cols per block in a chunk tile

    with tile.TileContext(nc) as tc:
        with (
            tc.tile_pool(name="pool", bufs=1) as pool,
            tc.tile_pool(name="psum", bufs=1, space="PSUM") as psum,
        ):
            bmh_t = pool.tile([128, KK * BS], f16, name="bmh")
            bml_t = pool.tile([128, KK * BS], f16, name="bml")
            nc.sync.dma_start(bmh_t[:], bmh_d.ap())
            nc.sync.dma_start(bml_t[:], bml_d.ap())

            xhc = [pool.tile([128, NB * CW], f16, name=f"xh{r}")
                   for r in range(RING)]
            xlc = [pool.tile([128, NB * CW], f16, name=f"xl{r}")
                   for r in range(RING)]
            # block 4 reads rows 488..518 only; zero its region once — DMAs
            # overwrite [0:30, block4] but never [30:, block4], which stays
            # zero across ring reuse. Ring-order + alternating engines so
            # chunk-0 DMAs aren't stuck behind 12 serial Pool memsets.
            for r in range(RING):
                nc.vector.memset(xhc[r][:, 4 * CW:5 * CW], 0.0)
                nc.gpsimd.memset(xlc[r][:, 4 * CW:5 * CW], 0.0)

            u_t = pool.tile([128, NB * 64], f32, name="u")
            v_t = pool.tile([128, NB * 64], f32, name="v")
            j_t = pool.tile([128, NB * 64], f32, name="j")
            nc.vector.memset(v_t[:], 0.0)
            nc.vector.memset(j_t[:], 0.0)

            mg = [pool.tile([128, GB * NB * 64], bf16, name=f"mg{i}")
                  for i in range(2)]
            pss = [psum.tile([128, NB * 64], f32, name=f"ps{i}")
                   for i in range(8)]

            in_eng = [nc.sync, nc.scalar]
            n_in = [0]

            def load_chunk(ci):
                r = ci % RING
                sz = CHUNKS[ci]
                t0 = CSTART[ci]
                for dst, src in ((xhc[r], xh_d), (xlc[r], xl_d)):
                    for b in range(NB):
                        nrows = min(128, HP - BS * b)
                        eng = in_eng[n_in[0] % len(in_eng)]
                        n_in[0] += 1
                        eng.dma_start(
                            dst[0:nrows, b * CW:b * CW + sz * KP]
                            .rearrange("p (t q) -> p t q", q=KP),
                            src.ap()[BS * b:BS * b + nrows, t0:t0 + sz, :])

            for ci in range(3):
                load_chunk(ci)

            # PE pstate warmup: harmless self-matmuls while preload streams in
            for i in range(16):
                nc.tensor.matmul(pss[7][0:BS, :], bmh_t[:, 0:BS],
                                 bmh_t[:, 0:320], start=True, stop=True)

            step2chunk = []
            for ci, sz in enumerate(CHUNKS):
                step2chunk += [(ci, tl) for tl in range(sz)]

            for t in range(T):
                ck, tl = step2chunk[t]
                if tl == 0 and ck + 3 < len(CHUNKS):
                    load_chunk(ck + 3)
                r = ck % RING
                mvh = xhc[r][:, :].rearrange(
                    "p (b t q) -> p b t q", b=NB, t=CK)
                mvl = xlc[r][:, :].rearrange(
                    "p (b t q) -> p b t q", b=NB, t=CK)
                ps = pss[t % 8]
                n = 0
                for dx in range(KK):
                    for bm_t, mv in ((bmh_t, mvh), (bmh_t, mvl),
                                     (bml_t, mvh)):
                        nc.tensor.matmul(
                            ps[0:BS, :],
                            bm_t[:, dx * BS:(dx + 1) * BS],
                            mv[:, :, tl:tl + 1, dx:dx + 64],
                            start=(n == 0), stop=(n == 3 * KK - 1),
                        )
                        n += 1

                msl = mg[(t // GB) % 2][0:BS, (t % GB) * 320:(t % GB + 1) * 320]
                nc.vector.scalar_tensor_tensor(
                    u_t[0:BS, :], v_t[0:BS, :], 0.9, j_t[0:BS, :],
                    Alu.mult, Alu.add)
                nc.vector.tensor_scalar(
                    msl, u_t[0:BS, :], 1.0, None, Alu.is_le)
                nc.vector.tensor_tensor(
                    v_t[0:BS, :], u_t[0:BS, :], msl, Alu.mult)
                nc.vector.scalar_tensor_tensor(
                    j_t[0:BS, :], j_t[0:BS, :], 0.9, ps[0:BS, :],
                    Alu.mult, Alu.add)

                if t % GB == GB - 1:
                    g = t // GB
                    nc.gpsimd.dma_start(ms_d.ap()[g], mg[g % 2][0:BS, :])

    nc.compile()
    _cached = nc
    return nc


def _bands(Wq):
    bm = np.zeros((128, KK * BS), np.float32)
    m = np.arange(BS)
    for dx in range(KK):
        for dy in range(KK):
            bm[m + dy, dx * BS + m] = Wq[dy, dx]
    return bm.astype(np.float16)


def kernel(x, W):
    x = np.asarray(x, np.float32)
    nc = _build()

    W01 = (np.float32(0.1) * np.asarray(W, np.float32).reshape(KK, KK))
    wh = W01.astype(np.float16)
    wl = (W01 - wh.astype(np.float32)).astype(np.float16)
    bmh = _bands(wh.astype(np.float32))
    bml = _bands(wl.astype(np.float32))

    in_maps = []
    for c in range(NCORES):
        slab = np.zeros((T, HP, KP), np.float32)
        lo = COLS * c - PAD
        s0, s1 = max(0, lo), min(WD, lo + KP)
        slab[:, PAD:PAD + H, s0 - lo:s0 - lo + s1 - s0] = x[:, 0, :, s0:s1]
        xs = np.ascontiguousarray(slab.transpose(1, 0, 2))   # [518, 128, 70]
        xh = xs.astype(np.float16)
        xl = (xs - xh.astype(np.float32)).astype(np.float16)
        in_maps.append({"xh": xh, "xl": xl, "bmh": bmh, "bml": bml})

    res = run_bass_kernel_spmd(nc, in_maps, core_ids=list(range(NCORES)))

    z = np.empty((T, H, WD), np.float32)
    for c in range(NCORES):
        ms = np.asarray(res.results[c]["ms"]).astype(np.float32)
        # [NG, BS, GB*NB*64] -> [t, block, row, w]
        m4 = ms.reshape(NG, BS, GB, NB, 64).transpose(0, 2, 3, 1, 4)
        m4 = m4.reshape(T, NB, BS, 64)
        zc = z[:, :, COLS * c:COLS * (c + 1)]
        zc[:, 0:4 * BS, :] = m4[:, 0:4].reshape(T, 4 * BS, 64)
        zc[:, 4 * BS:H, :] = m4[:, 4, 0:H - 4 * BS, :]
    return (np.float32(1.0) - z).reshape(T, 1, H, WD)
